# revision 59
# baseline (speedup 1.0000x reference)
"""Trainium2 Bass kernel for 16-head MHA (E=1024, S=2048, B=4) on 8 NeuronCores.

Sharding: tensor-parallel over head groups (TP=2: heads 0-7 / 8-15) x
data-parallel over batch (DP=4).  Core c handles batch c//2, head group c%2.
Host sums the two TP out-projection partials and adds b_out.

Device dataflow per core (per-stage dtypes chosen from an error budget):
  qk proj : bf16 matmuls -> PSUM; PSUM->SBUF copies emit a DITHER PAIR of
            e4m3 tensors q*4(1+h) / q*4(1-h), h=2^-5 (bias fused per-partition).
  v proj  : bf16 matmuls in transposed layout [vcol, t] (bias per-partition),
            fp16 result DMA-transposed into vaug [t, ti, head, 64+ones].
  scores  : fp8e4m3 DoubleRow matmuls, the 2 DR k-subtiles = the dither pair;
            first-order quantization error cancels.  out = scores^T [t, s].
  exp     : ACT exact exp->fp16 for most t-tiles; DVE Schraudolph
            (rint(z*c1+c2) as int16 bits == fp16 exp) for the rest.
  attn@V  : fp16 matmuls, et stationary [t, s-chunk], V moving [t, 64+1];
            out [s,65] accumulates over t in 4-chain PSUM banks; col 64 is
            the softmax denominator (ones column of vaug).
  norm    : per-partition reciprocal + tensor_scalar mult -> O bf16 [s, hd],
            head pairs packed [128 s, 128 hd], DMA-transposed into ot blocks.
  out proj: bf16 matmuls, ot stationary [hd, s-tile], W_out moving -> out
            [s, e] bf16, DMA'd per s-tile.
"""

import numpy as np
import ml_dtypes

import concourse.bass as bass
import concourse.tile as tile
from concourse import bacc, mybir
from concourse.alu_op_type import AluOpType
from concourse.bass_utils import run_bass_kernel_spmd

F32 = mybir.dt.float32
F16 = mybir.dt.float16
BF16 = mybir.dt.bfloat16
FP8 = mybir.dt.float8e4
I16 = mybir.dt.int16
EXP = mybir.ActivationFunctionType.Exp
IDENT = mybir.ActivationFunctionType.Identity
COPY = mybir.ActivationFunctionType.Copy
DR = mybir.MatmulPerfMode.DoubleRow

E = 1024          # embed dim
S = 2048          # sequence
B = 4             # batch
NH = 16           # total heads
HD = 64           # head dim
TP = 2            # head-group shards
HPC = NH // TP    # heads per core = 8
VW = HPC * HD     # 512 v columns per core
KCH = E // 128    # 8 contraction chunks

DITH = 2.0 ** -5                      # dither half-width
SP_, SM_ = 4.0 * (1 + DITH), 4.0 * (1 - DITH)
ESC = 1.0 / (256.0 * (1.0 + DITH * DITH))   # exp input scale (incl 1/sqrt(64))
SC1 = 1024.0 / np.log(2.0)            # fp16 schraudolph mult
SC2 = 15.0 * 1024.0 - 44.0            # fp16 schraudolph offset (tuned)
# exp engine per t-tile index: True -> DVE schraudolph, False -> ACT exact.
# Early units have PE-filler so ACT leads; late units split closer to even.
EXPMAP_EARLY = [False, False, True, False, False, True, False, True] * 2
EXPMAP_HEAD = [False, False, False, True, False, False, False, True] * 2
EXPMAP_LATE = [False, True, False, True, False, True, False, True] * 2

_CACHE = {}


def build_nc(debug_dump=False):
    nc = bacc.Bacc("TRN2", target_bir_lowering=False, debug=False,
                   num_devices=8)

    xb_d = nc.dram_tensor("xb", [128, KCH, S], BF16, kind="ExternalInput").ap()
    wqk_d = nc.dram_tensor("wqk", [128, 8, KCH, 128], BF16,
                           kind="ExternalInput").ap()
    bqkp_d = nc.dram_tensor("bqkp", [128, 8], F32, kind="ExternalInput").ap()
    bqkm_d = nc.dram_tensor("bqkm", [128, 8], F32, kind="ExternalInput").ap()
    wv_d = nc.dram_tensor("wv", [128, KCH, VW], BF16,
                          kind="ExternalInput").ap()
    bv_d = nc.dram_tensor("bv", [128, 4], F32, kind="ExternalInput").ap()
    wo_d = nc.dram_tensor("wo", [128, 4, E], BF16, kind="ExternalInput").ap()
    out_d = nc.dram_tensor("out", [S, E], BF16, kind="ExternalOutput").ap()
    if debug_dump:
        dbg = {n: nc.dram_tensor(n, shp, dt, kind="ExternalOutput").ap()
               for n, shp, dt in [
                   ("d_kt0", [128, 2, S], FP8), ("d_qt0", [128, 2, S], FP8),
                   ("d_vaug", [128, 16, HPC, 65], F16),
                   ("d_ot0", [128, S], BF16), ("d_ot3", [128, S], BF16),
                   ("d_et0", [128, 1024], F16), ("d_et2", [128, 1024], F16),
                   ("d_oa0", [128, 65], F32)]}

    with tile.TileContext(nc) as tc:
        xps_box = []
        with (tc.tile_pool(name="persist", bufs=1) as pp,
              tc.tile_pool(name="scps", bufs=2, space="PSUM") as scps,
              tc.tile_pool(name="oaps", bufs=2, space="PSUM") as oaps,
              tc.tile_pool(name="etp", bufs=26) as etp,
              tc.tile_pool(name="vst", bufs=2) as vst,
              tc.tile_pool(name="opr", bufs=2) as opr,
              tc.tile_pool(name="rcpp", bufs=4) as rcpp,
              tc.tile_pool(name="outp", bufs=3) as outp):
            bps = tc.alloc_tile_pool(name="bps", bufs=2, space="PSUM")
            xb = pp.tile([128, KCH, S], BF16, tag="xb")
            wqk = pp.tile([128, 8, KCH, 128], BF16, tag="wqk")
            bqkp = pp.tile([128, 8], F32, tag="bqkp")
            bqkm = pp.tile([128, 8], F32, tag="bqkm")
            wv = pp.tile([128, KCH, VW], BF16, tag="wv")
            bv = pp.tile([128, 4], F32, tag="bv")
            wo = pp.tile([128, 4, E], BF16, tag="wo")
            vaug = pp.tile([128, 16, HPC, 65], F16, tag="vaug")
            kt = [pp.tile([128, 2, S], FP8, tag=f"kt{i}", name=f"kt{i}")
                  for i in range(4)]
            qt = [pp.tile([128, 2, S], FP8, tag=f"qt{i}", name=f"qt{i}")
                  for i in range(4)]
            ot = [pp.tile([128, S], BF16, tag=f"ot{i}", name=f"ot{i}")
                  for i in range(4)]

            nc.vector.memset(vaug[:, :, :, 64:65], 1.0)

            # ---- input DMAs (order matters: m=4, m=0, wv/x early) ----
            morder = [4, 0, 5, 1, 6, 2, 7, 3]
            nc.sync.dma_start(wqk[:, 4, :, :], wqk_d[:, 4, :, :])
            nc.scalar.dma_start(bqkp[:], bqkp_d[:])
            nc.scalar.dma_start(bqkm[:], bqkm_d[:])
            for k in range(KCH):
                eng = nc.sync if k % 2 == 0 else nc.scalar
                eng.dma_start(xb[:, k, :], xb_d[:, k, :])
            nc.scalar.dma_start(wqk[:, 0, :, :], wqk_d[:, 0, :, :])
            nc.sync.dma_start(bv[:], bv_d[:])
            nc.sync.dma_start(wv[:], wv_d[:])
            for m in morder[2:]:
                eng = nc.sync if m % 2 == 0 else nc.scalar
                eng.dma_start(wqk[:, m, :, :], wqk_d[:, m, :, :])
            nc.scalar.dma_start(wo[:], wo_d[:])

            def act_recip(out, in_):
                eng = nc.scalar
                ins = [eng.lower_ap(in_)]
                for arg in (0.0, 1.0, 0.0):
                    ins.append(mybir.ImmediateValue(dtype=mybir.dt.float32,
                                                    value=arg))
                eng.add_instruction(mybir.InstActivation(
                    name=nc.get_next_instruction_name(),
                    func=mybir.ActivationFunctionType.Reciprocal,
                    ins=ins, outs=[eng.lower_ap(out)]))

            # ---- qk projection: m-tile -> dither pair in kt/qt ----
            def emit_qk(m, jlist=None):
                dest = kt[m - 4] if m >= 4 else qt[m]
                for j4 in jlist if jlist is not None else range(4):
                    ps = bps.tile([128, 512], F32, tag="ps",
                                  name=f"ps{m}_{j4}")
                    for k in range(KCH):
                        nc.tensor.matmul(
                            ps[:], wqk[:, m, k, :],
                            xb[:, k, j4 * 512:(j4 + 1) * 512],
                            start=(k == 0), stop=(k == KCH - 1))
                    sl = dest[:, 0, j4 * 512:(j4 + 1) * 512]
                    nc.vector.tensor_scalar(sl, ps[:], SP_, bqkp[:, m:m + 1],
                                            op0=AluOpType.mult,
                                            op1=AluOpType.add)
                    sl = dest[:, 1, j4 * 512:(j4 + 1) * 512]
                    nc.scalar.activation(sl, ps[:], IDENT,
                                         bias=bqkm[:, m:m + 1], scale=SM_)

            # ---- v projection (transposed layout), one vt = 2 heads ----
            def emit_v_quarter(vt, tc4):
                vsb = vst.tile([128, 512], F16, tag="vsb",
                               name=f"vsb{vt}_{tc4}")
                vp = bps.tile([128, 512], F32, tag="ps",
                              name=f"vp{vt}_{tc4}")
                for k in range(KCH):
                    nc.tensor.matmul(
                        vp[:], wv[:, k, vt * 128:(vt + 1) * 128],
                        xb[:, k, tc4 * 512:(tc4 + 1) * 512],
                        start=(k == 0), stop=(k == KCH - 1))
                nc.vector.tensor_scalar(
                    vsb[:], vp[:], bv[:, vt:vt + 1], None,
                    op0=AluOpType.add)
                sc4 = vst.tile([128, 4, 128], F16, tag="sc4",
                               name=f"sc4_{vt}_{tc4}")
                nc.sync.dma_start_transpose(sc4[:], vsb[:])
                nc.vector.tensor_copy(
                    vaug[:, tc4 * 4:(tc4 + 1) * 4, 2 * vt:2 * vt + 2, 0:64],
                    sc4[:].rearrange("p a (b c) -> p a b c", b=2))

            def emit_v(vt):
                for tc4 in range(4):
                    emit_v_quarter(vt, tc4)

            # preamble: minimal prefix for unit (h0, jj0); m4/m0 chains
            # interleave per-k so both finish as the xb DMA stream lands.
            def emit_qk_pair(j4):
                pa = bps.tile([128, 512], F32, tag="ps", name=f"pp4_{j4}")
                pb = bps.tile([128, 512], F32, tag="ps", name=f"pp0_{j4}")
                for k in range(KCH):
                    for m, ps in ((4, pa), (0, pb)):
                        nc.tensor.matmul(
                            ps[:], wqk[:, m, k, :],
                            xb[:, k, j4 * 512:(j4 + 1) * 512],
                            start=(k == 0), stop=(k == KCH - 1))
                for m, ps in ((4, pa), (0, pb)):
                    dest = kt[0] if m == 4 else qt[0]
                    sl = dest[:, 0, j4 * 512:(j4 + 1) * 512]
                    nc.vector.tensor_scalar(sl, ps[:], SP_, bqkp[:, m:m + 1],
                                            op0=AluOpType.mult,
                                            op1=AluOpType.add)
                    sl = dest[:, 1, j4 * 512:(j4 + 1) * 512]
                    nc.scalar.activation(sl, ps[:], IDENT,
                                         bias=bqkm[:, m:m + 1], scale=SM_)

            emit_qk_pair(0)
            emit_qk_pair(1)
            emit_v_quarter(0, 0)
            emit_v_quarter(0, 1)
            filler = [lambda: emit_qk(4, [2]), lambda: emit_qk(0, [2]),
                      lambda: emit_qk(4, [3]), lambda: emit_v_quarter(0, 2),
                      lambda: emit_v_quarter(0, 3), lambda: emit_qk(0, [3])]
            for m, vt in [(5, None), (1, None), (None, 1), (6, None),
                          (2, None), (None, 2), (7, None), (3, None),
                          (None, 3)]:
                if m is not None:
                    filler += [lambda m=m, j=j: emit_qk(m, [j])
                               for j in range(4)]
                else:
                    filler += [lambda v=vt, t=t: emit_v_quarter(v, t)
                               for t in range(4)]

            def pop_filler():
                if filler:
                    filler.pop(0)()

            # ---- attention unit (h, jj): jj = s-half (1024 wide).
            # wave-0 V-matmuls interleave with wave-1 scores; wave-1
            # V-matmuls + per-st drain become "pending" closures popped
            # one per slot inside the NEXT unit's first scores loop, so
            # the exp engines never starve across unit boundaries.
            opair = {}

            def attn_unit(h, jj, pend_mm, pend_drain):
                hp, p0 = h // 2, (h % 2) * 64
                uidx = h * 2 + jj
                expmap = EXPMAP_EARLY if uidx < 10 else EXPMAP_LATE
                oa = [None, None]
                ets = [None] * 16

                def get_oa(g):
                    if oa[g] is None:
                        oa[g] = oaps.tile([128, 4, 65], F32, tag="oa",
                                          name=f"oa{h}_{jj}_{g}")
                    return oa[g]

                def scores_one(ti):
                    if xps_box and ti % 3 == 2:
                        sc = xps_box[0].tile([128, 1024], F32, tag="xsc",
                                             name=f"sc{h}_{jj}_{ti}")
                    else:
                        sc = scps.tile([128, 1024], F32, tag="sc",
                                       name=f"sc{h}_{jj}_{ti}")
                    for sh in range(2):
                        nc.tensor.matmul(
                            sc[:, sh * 512:(sh + 1) * 512],
                            kt[hp][p0:p0 + 64, :, ti * 128:(ti + 1) * 128],
                            qt[hp][p0:p0 + 64, :,
                                   jj * 1024 + sh * 512:
                                   jj * 1024 + (sh + 1) * 512],
                            start=True, stop=True, perf_mode=DR)
                    et = etp.tile([128, 1024], F16, tag="et",
                                  name=f"et{h}_{jj}_{ti}")
                    if expmap[ti]:
                        nc.vector.tensor_scalar(
                            et[:].bitcast(I16), sc[:], ESC * SC1, SC2,
                            op0=AluOpType.mult, op1=AluOpType.add)
                    else:
                        nc.scalar.activation(et[:], sc[:], EXP, scale=ESC)
                    if debug_dump and h == 0 and jj == 0 and ti in (0, 2):
                        nc.sync.dma_start(dbg[f"d_et{ti}"][:], et[:])
                    ets[ti] = et

                def vmm(st, ti):
                    nc.tensor.matmul(
                        get_oa(st // 4)[:, st % 4, :],
                        ets[ti][:, st * 128:(st + 1) * 128],
                        vaug[:, ti, h, :],
                        start=(ti == 0), stop=(ti == 15))

                for tl in range(8):
                    scores_one(tl)
                    if pend_mm:
                        pend_mm.pop(0)()
                    if tl in (2, 5) and uidx < 10:
                        pop_filler()
                for tl in range(8):
                    scores_one(8 + tl)
                    if pend_drain:
                        pend_drain.pop(0)()
                    if ((tl in (2, 4, 5, 7) and uidx == 0) or
                            (tl in (2, 5) and 0 < uidx < 10)):
                        pop_filler()
                    if uidx == 15 and tl >= 4:
                        emit_out(tl - 4)

                if (hp, jj) not in opair:
                    opair[(hp, jj)] = opr.tile([128, 8, 128], BF16, tag="op",
                                               name=f"op{hp}_{jj}")
                osb = opair[(hp, jj)]

                def piece_mm(st):
                    for tl in range(16):
                        vmm(st, tl)
                    if debug_dump and h == 0 and jj == 0 and st == 0:
                        dtmp = vst.tile([128, 65], F32, tag="dtmp")
                        nc.vector.tensor_copy(dtmp[:], oa[0][:, 0, :])
                        nc.sync.dma_start(dbg["d_oa0"][:], dtmp[:])

                def piece_drain(st):
                    rcp = rcpp.tile([128, 1], F32, tag="rcp")
                    nc.vector.reciprocal(rcp[:], oa[st // 4][:, st % 4,
                                                             64:65])
                    dst = osb[:, st, (h % 2) * 64:(h % 2) * 64 + 64]
                    src = oa[st // 4][:, st % 4, 0:64]
                    if st % 2 == 0 and uidx < 10:
                        nc.vector.tensor_scalar(dst, src, rcp[:], None,
                                                op0=AluOpType.mult)
                    else:
                        nc.scalar.activation(dst, src, COPY, scale=rcp[:])
                    if h % 2 == 1 and st in (3, 7):
                        half = osb[:, st - 3:st + 1, :]
                        nc.sync.dma_start_transpose(
                            ot[hp][:, jj * 1024 + (st - 3) * 128:
                                   jj * 1024 + (st + 1) * 128].rearrange(
                                "p (a c) -> p a c", c=128),
                            half.rearrange("p a c -> p (a c)"))
                        if st == 7:
                            del opair[(hp, jj)]

                return ([lambda st=st: piece_mm(st) for st in range(8)],
                        [lambda st=st: piece_drain(st) for st in range(8)])

            # ---- out projection for one s-tile ----
            def emit_out(st):
                osb = outp.tile([128, E], BF16, tag="outsb", name=f"os{st}")
                pool = scps if st % 2 == 0 else xps_box[0]
                tag = "sc" if st % 2 == 0 else "xsc"
                op = pool.tile([128, 1024], F32, tag=tag, name=f"op{st}")
                for e2 in range(2):
                    for kc in range(4):
                        nc.tensor.matmul(
                            op[:, e2 * 512:(e2 + 1) * 512],
                            ot[kc][:, st * 128:(st + 1) * 128],
                            wo[:, kc, e2 * 512:(e2 + 1) * 512],
                            start=(kc == 0), stop=(kc == 3))
                if st % 2 == 0:
                    nc.scalar.activation(osb[:], op[:], COPY)
                else:
                    nc.vector.tensor_copy(osb[:], op[:])
                nc.sync.dma_start(out_d[st * 128:(st + 1) * 128, :],
                                  osb[:])

            # ---- main schedule: h-outer, projections drip-fed as filler,
            # per-unit trailing work software-pipelined via `pending`.
            pend_mm, pend_drain = [], []
            for h in range(HPC):
                for jj in range(2):
                    if h * 2 + jj == 10:
                        bps.release()
                        xps_box.append(tc.alloc_tile_pool(
                            name="xps", bufs=1, space="PSUM"))
                    pend_mm, pend_drain = attn_unit(h, jj, pend_mm,
                                                    pend_drain)
            while filler:
                filler.pop(0)()
            for st in range(4, 8):
                emit_out(st)
            for i in range(8):
                pend_mm.pop(0)()
            for i in range(8):
                pend_drain.pop(0)()
                if i == 3:
                    for st in range(8, 12):
                        emit_out(st)
            for st in range(12, 16):
                emit_out(st)
            if debug_dump:
                nc.sync.dma_start(dbg["d_kt0"][:], kt[0][:])
                nc.sync.dma_start(dbg["d_qt0"][:], qt[0][:])
                nc.sync.dma_start(dbg["d_vaug"][:], vaug[:])
                nc.sync.dma_start(dbg["d_ot0"][:], ot[0][:])
                nc.sync.dma_start(dbg["d_ot3"][:], ot[3][:])
            xps_box[0].release()

    nc.compile()
    return nc


def _shard_inputs(x, W_qkv, b_qkv, W_out, b_out):
    BF = ml_dtypes.bfloat16
    xbs = []
    for b in range(B):
        xT = np.ascontiguousarray(x[b].T)                       # [E, S]
        xbs.append(np.ascontiguousarray(
            xT.reshape(KCH, 128, S).transpose(1, 0, 2)).astype(BF))
    gshards = []
    for g in range(TP):
        lo, hi = g * VW, (g + 1) * VW
        Wq = W_qkv[:, lo:hi]
        Wk = W_qkv[:, E + lo:E + hi]
        Wv_ = W_qkv[:, 2 * E + lo:2 * E + hi]
        bq = b_qkv[lo:hi]
        bk = b_qkv[E + lo:E + hi]
        bvv = b_qkv[2 * E + lo:2 * E + hi]
        Wqk = np.concatenate([Wq, Wk], axis=1)                  # [E, 1024]
        wqk = np.ascontiguousarray(
            Wqk.reshape(KCH, 128, 8, 128).transpose(1, 2, 0, 3)).astype(BF)
        bcat = np.concatenate([bq, bk]).reshape(8, 128).T       # [128, 8]
        wv = np.ascontiguousarray(
            Wv_.reshape(KCH, 128, VW).transpose(1, 0, 2)).astype(BF)
        bvt = bvv.reshape(4, 128).T                             # [128, 4]
        wo = np.ascontiguousarray(
            W_out[lo:hi, :].reshape(4, 128, E).transpose(1, 0, 2)).astype(BF)
        gshards.append({
            "wqk": wqk,
            "bqkp": np.ascontiguousarray(bcat * SP_, dtype=np.float32),
            "bqkm": np.ascontiguousarray(bcat * SM_, dtype=np.float32),
            "wv": wv,
            "bv": np.ascontiguousarray(bvt, dtype=np.float32),
            "wo": wo,
        })
    in_maps = []
    for c in range(8):
        b, g = c // TP, c % TP
        m = dict(gshards[g])
        m["xb"] = xbs[b]
        in_maps.append(m)
    return in_maps


def kernel(x, W_qkv, b_qkv, W_out, b_out):
    x = np.asarray(x, dtype=np.float32)
    W_qkv = np.asarray(W_qkv, dtype=np.float32)
    b_qkv = np.asarray(b_qkv, dtype=np.float32)
    W_out = np.asarray(W_out, dtype=np.float32)
    b_out = np.asarray(b_out, dtype=np.float32)
    if "nc" not in _CACHE:
        _CACHE["nc"] = build_nc()
    nc = _CACHE["nc"]
    in_maps = _shard_inputs(x, W_qkv, b_qkv, W_out, b_out)
    res = None
    for attempt in range(3):
        try:
            res = run_bass_kernel_spmd(nc, in_maps, core_ids=list(range(8)))
            break
        except Exception:
            if attempt == 2:
                raise
    _CACHE["last_results"] = res
    out = np.empty((B, S, E), dtype=np.float32)
    for b in range(B):
        out[b] = (res.results[TP * b]["out"].astype(np.float32) +
                  res.results[TP * b + 1]["out"].astype(np.float32) + b_out)
    return out


# revision 60
# speedup vs baseline: 1.0414x; 1.0414x over previous
"""Trainium2 Bass kernel for 16-head MHA (E=1024, S=2048, B=4) on 8 NeuronCores.

Sharding: tensor-parallel over head groups (TP=2: heads 0-7 / 8-15) x
data-parallel over batch (DP=4).  Core c handles batch c//2, head group c%2.
Host sums the two TP out-projection partials and adds b_out.

Device dataflow per core (per-stage dtypes chosen from an error budget):
  qk proj : bf16 matmuls -> PSUM; PSUM->SBUF copies emit a DITHER PAIR of
            e4m3 tensors q*4(1+h) / q*4(1-h), h=2^-5 (bias fused per-partition).
  v proj  : bf16 matmuls in transposed layout [vcol, t] (bias per-partition),
            fp16 result DMA-transposed into vaug [t, ti, head, 64+ones].
  scores  : fp8e4m3 DoubleRow matmuls, the 2 DR k-subtiles = the dither pair;
            first-order quantization error cancels.  out = scores^T [t, s].
  exp     : ACT exact exp->fp16 for most t-tiles; DVE Schraudolph
            (rint(z*c1+c2) as int16 bits == fp16 exp) for the rest.
  attn@V  : fp16 matmuls, et stationary [t, s-chunk], V moving [t, 64+1];
            out [s,65] accumulates over t in 4-chain PSUM banks; col 64 is
            the softmax denominator (ones column of vaug).
  norm    : per-partition reciprocal + tensor_scalar mult -> O bf16 [s, hd],
            head pairs packed [128 s, 128 hd], DMA-transposed into ot blocks.
  out proj: bf16 matmuls, ot stationary [hd, s-tile], W_out moving -> out
            [s, e] bf16, DMA'd per s-tile.
"""

import numpy as np
import ml_dtypes

import concourse.bass as bass
import concourse.tile as tile
from concourse import bacc, mybir
from concourse.alu_op_type import AluOpType
from concourse.bass_utils import run_bass_kernel_spmd

F32 = mybir.dt.float32
F16 = mybir.dt.float16
BF16 = mybir.dt.bfloat16
FP8 = mybir.dt.float8e4
I16 = mybir.dt.int16
EXP = mybir.ActivationFunctionType.Exp
IDENT = mybir.ActivationFunctionType.Identity
COPY = mybir.ActivationFunctionType.Copy
DR = mybir.MatmulPerfMode.DoubleRow

E = 1024          # embed dim
S = 2048          # sequence
B = 4             # batch
NH = 16           # total heads
HD = 64           # head dim
TP = 2            # head-group shards
HPC = NH // TP    # heads per core = 8
VW = HPC * HD     # 512 v columns per core
KCH = E // 128    # 8 contraction chunks

DITH = 2.0 ** -5                      # dither half-width
SP_, SM_ = 4.0 * (1 + DITH), 4.0 * (1 - DITH)
ESC = 1.0 / (256.0 * (1.0 + DITH * DITH))   # exp input scale (incl 1/sqrt(64))
SC1 = 1024.0 / np.log(2.0)            # fp16 schraudolph mult
SC2 = 15.0 * 1024.0 - 44.0            # fp16 schraudolph offset (tuned)
# exp engine per t-tile index: True -> DVE schraudolph, False -> ACT exact.
# Early units have PE-filler so ACT leads; late units split closer to even.
EXPMAP_EARLY = [False, False, True, False, False, True, False, True] * 2
EXPMAP_HEAD = [False, False, False, True, False, False, False, True] * 2
EXPMAP_LATE = [False, True, False, True, False, True, False, True] * 2

_CACHE = {}


def build_nc(debug_dump=False):
    nc = bacc.Bacc("TRN2", target_bir_lowering=False, debug=False,
                   num_devices=8)

    xb_d = nc.dram_tensor("xb", [128, KCH, S], BF16, kind="ExternalInput").ap()
    wqk_d = nc.dram_tensor("wqk", [128, 8, KCH, 128], BF16,
                           kind="ExternalInput").ap()
    bqkp_d = nc.dram_tensor("bqkp", [128, 8], F32, kind="ExternalInput").ap()
    bqkm_d = nc.dram_tensor("bqkm", [128, 8], F32, kind="ExternalInput").ap()
    wv_d = nc.dram_tensor("wv", [128, KCH, VW], BF16,
                          kind="ExternalInput").ap()
    bv_d = nc.dram_tensor("bv", [128, 4], F32, kind="ExternalInput").ap()
    wo_d = nc.dram_tensor("wo", [128, 4, E], BF16, kind="ExternalInput").ap()
    out_d = nc.dram_tensor("out", [S, E], BF16, kind="ExternalOutput").ap()
    if debug_dump:
        dbg = {n: nc.dram_tensor(n, shp, dt, kind="ExternalOutput").ap()
               for n, shp, dt in [
                   ("d_kt0", [128, 2, S], FP8), ("d_qt0", [128, 2, S], FP8),
                   ("d_vaug", [128, 16, HPC, 65], F16),
                   ("d_ot0", [128, S], BF16), ("d_ot3", [128, S], BF16),
                   ("d_et0", [128, 1024], F16), ("d_et2", [128, 1024], F16),
                   ("d_oa0", [128, 65], F32)]}

    with tile.TileContext(nc) as tc:
        xps_box = []
        with (tc.tile_pool(name="persist", bufs=1) as pp,
              tc.tile_pool(name="scps", bufs=2, space="PSUM") as scps,
              tc.tile_pool(name="oaps", bufs=2, space="PSUM") as oaps,
              tc.tile_pool(name="etp", bufs=26) as etp,
              tc.tile_pool(name="vst", bufs=2) as vst,
              tc.tile_pool(name="opr", bufs=2) as opr,
              tc.tile_pool(name="rcpp", bufs=4) as rcpp,
              tc.tile_pool(name="outp", bufs=3) as outp):
            bps = tc.alloc_tile_pool(name="bps", bufs=2, space="PSUM")
            xb = pp.tile([128, KCH, S], BF16, tag="xb")
            wqk = pp.tile([128, 8, KCH, 128], BF16, tag="wqk")
            bqkp = pp.tile([128, 8], F32, tag="bqkp")
            bqkm = pp.tile([128, 8], F32, tag="bqkm")
            wv = pp.tile([128, KCH, VW], BF16, tag="wv")
            bv = pp.tile([128, 4], F32, tag="bv")
            wo = pp.tile([128, 4, E], BF16, tag="wo")
            vaug = pp.tile([128, 16, HPC, 65], F16, tag="vaug")
            kt = [pp.tile([128, 2, S], FP8, tag=f"kt{i}", name=f"kt{i}")
                  for i in range(4)]
            qt = [pp.tile([128, 2, S], FP8, tag=f"qt{i}", name=f"qt{i}")
                  for i in range(4)]
            ot = [pp.tile([128, S], BF16, tag=f"ot{i}", name=f"ot{i}")
                  for i in range(4)]

            nc.vector.memset(vaug[:, :, :, 64:65], 1.0)

            # ---- input DMAs (order matters: m=4, m=0, wv/x early) ----
            morder = [4, 0, 5, 1, 6, 2, 7, 3]
            nc.sync.dma_start(wqk[:, 4, :, :], wqk_d[:, 4, :, :])
            nc.scalar.dma_start(bqkp[:], bqkp_d[:])
            nc.scalar.dma_start(bqkm[:], bqkm_d[:])
            for k in range(KCH):
                eng = nc.sync if k % 2 == 0 else nc.scalar
                eng.dma_start(xb[:, k, :], xb_d[:, k, :])
            nc.scalar.dma_start(wqk[:, 0, :, :], wqk_d[:, 0, :, :])
            nc.sync.dma_start(bv[:], bv_d[:])
            nc.sync.dma_start(wv[:], wv_d[:])
            for m in morder[2:]:
                eng = nc.sync if m % 2 == 0 else nc.scalar
                eng.dma_start(wqk[:, m, :, :], wqk_d[:, m, :, :])
            nc.scalar.dma_start(wo[:], wo_d[:])

            def act_recip(out, in_):
                eng = nc.scalar
                ins = [eng.lower_ap(in_)]
                for arg in (0.0, 1.0, 0.0):
                    ins.append(mybir.ImmediateValue(dtype=mybir.dt.float32,
                                                    value=arg))
                eng.add_instruction(mybir.InstActivation(
                    name=nc.get_next_instruction_name(),
                    func=mybir.ActivationFunctionType.Reciprocal,
                    ins=ins, outs=[eng.lower_ap(out)]))

            # ---- qk projection: m-tile -> dither pair in kt/qt ----
            def emit_qk(m, jlist=None):
                dest = kt[m - 4] if m >= 4 else qt[m]
                for j4 in jlist if jlist is not None else range(4):
                    ps = bps.tile([128, 512], F32, tag="ps",
                                  name=f"ps{m}_{j4}")
                    for k in range(KCH):
                        nc.tensor.matmul(
                            ps[:], wqk[:, m, k, :],
                            xb[:, k, j4 * 512:(j4 + 1) * 512],
                            start=(k == 0), stop=(k == KCH - 1))
                    sl = dest[:, 0, j4 * 512:(j4 + 1) * 512]
                    nc.vector.tensor_scalar(sl, ps[:], SP_, bqkp[:, m:m + 1],
                                            op0=AluOpType.mult,
                                            op1=AluOpType.add)
                    sl = dest[:, 1, j4 * 512:(j4 + 1) * 512]
                    nc.scalar.activation(sl, ps[:], IDENT,
                                         bias=bqkm[:, m:m + 1], scale=SM_)

            # ---- v projection (transposed layout), one vt = 2 heads ----
            def emit_v_quarter(vt, tc4):
                vsb = vst.tile([128, 512], F16, tag="vsb",
                               name=f"vsb{vt}_{tc4}")
                vp = bps.tile([128, 512], F32, tag="ps",
                              name=f"vp{vt}_{tc4}")
                for k in range(KCH):
                    nc.tensor.matmul(
                        vp[:], wv[:, k, vt * 128:(vt + 1) * 128],
                        xb[:, k, tc4 * 512:(tc4 + 1) * 512],
                        start=(k == 0), stop=(k == KCH - 1))
                nc.vector.tensor_scalar(
                    vsb[:], vp[:], bv[:, vt:vt + 1], None,
                    op0=AluOpType.add)
                sc4 = vst.tile([128, 4, 128], F16, tag="sc4",
                               name=f"sc4_{vt}_{tc4}")
                nc.sync.dma_start_transpose(sc4[:], vsb[:])
                nc.vector.tensor_copy(
                    vaug[:, tc4 * 4:(tc4 + 1) * 4, 2 * vt:2 * vt + 2, 0:64],
                    sc4[:].rearrange("p a (b c) -> p a b c", b=2))

            def emit_v(vt):
                for tc4 in range(4):
                    emit_v_quarter(vt, tc4)

            # preamble: minimal prefix for unit (h0, jj0) wave-0; the rest
            # drip-feeds as fine-grained filler popped inside attention units
            emit_qk(4)
            emit_qk(0, [0, 1])
            emit_v_quarter(0, 0)
            emit_v_quarter(0, 1)
            filler = [lambda: emit_v_quarter(0, 2), lambda: emit_v_quarter(0, 3),
                      lambda: emit_qk(0, [2]), lambda: emit_qk(0, [3])]
            for m, vt in [(5, None), (1, None), (None, 1), (6, None),
                          (2, None), (None, 2), (7, None), (3, None),
                          (None, 3)]:
                if m is not None:
                    filler += [lambda m=m, j=j: emit_qk(m, [j])
                               for j in range(4)]
                else:
                    filler += [lambda v=vt, t=t: emit_v_quarter(v, t)
                               for t in range(4)]

            def pop_filler():
                if filler:
                    filler.pop(0)()

            # ---- attention unit (h, jj): jj = s-half (1024 wide).
            # wave-0 V-matmuls interleave with wave-1 scores; wave-1
            # V-matmuls + per-st drain become "pending" closures popped
            # one per slot inside the NEXT unit's first scores loop, so
            # the exp engines never starve across unit boundaries.
            opair = {}

            def attn_unit(h, jj, pend_mm, pend_drain):
                hp, p0 = h // 2, (h % 2) * 64
                uidx = h * 2 + jj
                expmap = EXPMAP_EARLY if uidx < 10 else EXPMAP_LATE
                oa = [None, None]
                ets = [None] * 16

                def get_oa(g):
                    if oa[g] is None:
                        oa[g] = oaps.tile([128, 4, 65], F32, tag="oa",
                                          name=f"oa{h}_{jj}_{g}")
                    return oa[g]

                def scores_one(ti):
                    if xps_box and ti % 3 == 2:
                        sc = xps_box[0].tile([128, 1024], F32, tag="xsc",
                                             name=f"sc{h}_{jj}_{ti}")
                    else:
                        sc = scps.tile([128, 1024], F32, tag="sc",
                                       name=f"sc{h}_{jj}_{ti}")
                    for sh in range(2):
                        nc.tensor.matmul(
                            sc[:, sh * 512:(sh + 1) * 512],
                            kt[hp][p0:p0 + 64, :, ti * 128:(ti + 1) * 128],
                            qt[hp][p0:p0 + 64, :,
                                   jj * 1024 + sh * 512:
                                   jj * 1024 + (sh + 1) * 512],
                            start=True, stop=True, perf_mode=DR)
                    et = etp.tile([128, 1024], F16, tag="et",
                                  name=f"et{h}_{jj}_{ti}")
                    if expmap[ti]:
                        nc.vector.tensor_scalar(
                            et[:].bitcast(I16), sc[:], ESC * SC1, SC2,
                            op0=AluOpType.mult, op1=AluOpType.add)
                    else:
                        nc.scalar.activation(et[:], sc[:], EXP, scale=ESC)
                    if debug_dump and h == 0 and jj == 0 and ti in (0, 2):
                        nc.sync.dma_start(dbg[f"d_et{ti}"][:], et[:])
                    ets[ti] = et

                def vmm(st, ti):
                    nc.tensor.matmul(
                        get_oa(st // 4)[:, st % 4, :],
                        ets[ti][:, st * 128:(st + 1) * 128],
                        vaug[:, ti, h, :],
                        start=(ti == 0), stop=(ti == 15))

                for tl in range(8):
                    scores_one(tl)
                    if pend_mm:
                        pend_mm.pop(0)()
                    if tl in (2, 5) and uidx < 10:
                        pop_filler()
                for tl in range(8):
                    scores_one(8 + tl)
                    if pend_drain:
                        pend_drain.pop(0)()
                    if tl in (2, 5) and uidx < 10:
                        pop_filler()
                    if uidx == 15 and tl >= 4:
                        emit_out(tl - 4)

                if (hp, jj) not in opair:
                    opair[(hp, jj)] = opr.tile([128, 8, 128], BF16, tag="op",
                                               name=f"op{hp}_{jj}")
                osb = opair[(hp, jj)]

                def piece_mm(st):
                    for tl in range(16):
                        vmm(st, tl)
                    if debug_dump and h == 0 and jj == 0 and st == 0:
                        dtmp = vst.tile([128, 65], F32, tag="dtmp")
                        nc.vector.tensor_copy(dtmp[:], oa[0][:, 0, :])
                        nc.sync.dma_start(dbg["d_oa0"][:], dtmp[:])

                def piece_drain(st):
                    rcp = rcpp.tile([128, 1], F32, tag="rcp")
                    nc.vector.reciprocal(rcp[:], oa[st // 4][:, st % 4,
                                                             64:65])
                    dst = osb[:, st, (h % 2) * 64:(h % 2) * 64 + 64]
                    src = oa[st // 4][:, st % 4, 0:64]
                    if st % 2 == 0 and uidx < 10:
                        nc.vector.tensor_scalar(dst, src, rcp[:], None,
                                                op0=AluOpType.mult)
                    else:
                        nc.scalar.activation(dst, src, COPY, scale=rcp[:])
                    if h % 2 == 1 and st in (3, 7):
                        half = osb[:, st - 3:st + 1, :]
                        nc.sync.dma_start_transpose(
                            ot[hp][:, jj * 1024 + (st - 3) * 128:
                                   jj * 1024 + (st + 1) * 128].rearrange(
                                "p (a c) -> p a c", c=128),
                            half.rearrange("p a c -> p (a c)"))
                        if st == 7:
                            del opair[(hp, jj)]

                return ([lambda st=st: piece_mm(st) for st in range(8)],
                        [lambda st=st: piece_drain(st) for st in range(8)])

            # ---- out projection for one s-tile ----
            def emit_out(st):
                osb = outp.tile([128, E], BF16, tag="outsb", name=f"os{st}")
                pool = scps if st % 2 == 0 else xps_box[0]
                tag = "sc" if st % 2 == 0 else "xsc"
                op = pool.tile([128, 1024], F32, tag=tag, name=f"op{st}")
                for e2 in range(2):
                    for kc in range(4):
                        nc.tensor.matmul(
                            op[:, e2 * 512:(e2 + 1) * 512],
                            ot[kc][:, st * 128:(st + 1) * 128],
                            wo[:, kc, e2 * 512:(e2 + 1) * 512],
                            start=(kc == 0), stop=(kc == 3))
                if st % 2 == 0:
                    nc.scalar.activation(osb[:], op[:], COPY)
                else:
                    nc.vector.tensor_copy(osb[:], op[:])
                nc.sync.dma_start(out_d[st * 128:(st + 1) * 128, :],
                                  osb[:])

            # ---- main schedule: h-outer, projections drip-fed as filler,
            # per-unit trailing work software-pipelined via `pending`.
            pend_mm, pend_drain = [], []
            for h in range(HPC):
                for jj in range(2):
                    if h * 2 + jj == 10:
                        bps.release()
                        xps_box.append(tc.alloc_tile_pool(
                            name="xps", bufs=1, space="PSUM"))
                    pend_mm, pend_drain = attn_unit(h, jj, pend_mm,
                                                    pend_drain)
            while filler:
                filler.pop(0)()
            for st in range(4, 8):
                emit_out(st)
            for i in range(8):
                pend_mm.pop(0)()
            for i in range(8):
                pend_drain.pop(0)()
                if i == 3:
                    for st in range(8, 12):
                        emit_out(st)
            for st in range(12, 16):
                emit_out(st)
            if debug_dump:
                nc.sync.dma_start(dbg["d_kt0"][:], kt[0][:])
                nc.sync.dma_start(dbg["d_qt0"][:], qt[0][:])
                nc.sync.dma_start(dbg["d_vaug"][:], vaug[:])
                nc.sync.dma_start(dbg["d_ot0"][:], ot[0][:])
                nc.sync.dma_start(dbg["d_ot3"][:], ot[3][:])
            xps_box[0].release()

    nc.compile()
    return nc


def _shard_inputs(x, W_qkv, b_qkv, W_out, b_out):
    BF = ml_dtypes.bfloat16
    xbs = []
    for b in range(B):
        xT = np.ascontiguousarray(x[b].T)                       # [E, S]
        xbs.append(np.ascontiguousarray(
            xT.reshape(KCH, 128, S).transpose(1, 0, 2)).astype(BF))
    gshards = []
    for g in range(TP):
        lo, hi = g * VW, (g + 1) * VW
        Wq = W_qkv[:, lo:hi]
        Wk = W_qkv[:, E + lo:E + hi]
        Wv_ = W_qkv[:, 2 * E + lo:2 * E + hi]
        bq = b_qkv[lo:hi]
        bk = b_qkv[E + lo:E + hi]
        bvv = b_qkv[2 * E + lo:2 * E + hi]
        Wqk = np.concatenate([Wq, Wk], axis=1)                  # [E, 1024]
        wqk = np.ascontiguousarray(
            Wqk.reshape(KCH, 128, 8, 128).transpose(1, 2, 0, 3)).astype(BF)
        bcat = np.concatenate([bq, bk]).reshape(8, 128).T       # [128, 8]
        wv = np.ascontiguousarray(
            Wv_.reshape(KCH, 128, VW).transpose(1, 0, 2)).astype(BF)
        bvt = bvv.reshape(4, 128).T                             # [128, 4]
        wo = np.ascontiguousarray(
            W_out[lo:hi, :].reshape(4, 128, E).transpose(1, 0, 2)).astype(BF)
        gshards.append({
            "wqk": wqk,
            "bqkp": np.ascontiguousarray(bcat * SP_, dtype=np.float32),
            "bqkm": np.ascontiguousarray(bcat * SM_, dtype=np.float32),
            "wv": wv,
            "bv": np.ascontiguousarray(bvt, dtype=np.float32),
            "wo": wo,
        })
    in_maps = []
    for c in range(8):
        b, g = c // TP, c % TP
        m = dict(gshards[g])
        m["xb"] = xbs[b]
        in_maps.append(m)
    return in_maps


def kernel(x, W_qkv, b_qkv, W_out, b_out):
    x = np.asarray(x, dtype=np.float32)
    W_qkv = np.asarray(W_qkv, dtype=np.float32)
    b_qkv = np.asarray(b_qkv, dtype=np.float32)
    W_out = np.asarray(W_out, dtype=np.float32)
    b_out = np.asarray(b_out, dtype=np.float32)
    if "nc" not in _CACHE:
        _CACHE["nc"] = build_nc()
    nc = _CACHE["nc"]
    in_maps = _shard_inputs(x, W_qkv, b_qkv, W_out, b_out)
    res = None
    for attempt in range(3):
        try:
            res = run_bass_kernel_spmd(nc, in_maps, core_ids=list(range(8)))
            break
        except Exception:
            if attempt == 2:
                raise
    _CACHE["last_results"] = res
    out = np.empty((B, S, E), dtype=np.float32)
    for b in range(B):
        out[b] = (res.results[TP * b]["out"].astype(np.float32) +
                  res.results[TP * b + 1]["out"].astype(np.float32) + b_out)
    return out
